# revision 1
# baseline (speedup 1.0000x reference)
"""Multi-head causal attention (B=2, L=2048, D=1024, H=16) on 8 trn2 cores.

Sharding: core c -> batch b=c//4, head-group g=c%4 (4 heads / 256 of D).
Each core computes its Q/K/V projections in transposed layout, causal
attention with transposed scores (softmax denominators via a ones-column
appended to V, no max-subtraction needed: |scores/8| <~ 2), and a partial
output projection against its 256-row slice of w_o^T.  The 4 partials per
batch are summed on the host (+ b_o) during unsharding.
"""

import sys

sys.path.insert(0, "/opt/trn_rl_repo")

import numpy as np
import ml_dtypes

import concourse.bass as bass
import concourse.mybir as mybir
import concourse.tile as tile
from concourse.bass_utils import run_bass_kernel_spmd

BF16 = mybir.dt.bfloat16
F32 = mybir.dt.float32
F32R = mybir.dt.float32r

B, L, D, H = 2, 2048, 1024, 16
DK = 64            # head dim
HPC = 4            # heads per core
DS = HPC * DK      # 256: D-slice per core
KT = D // 128      # 8 k-tiles over D
N_CORES = 8
NCH = L // 512     # 4 q-chunks of 512
NSB = 2            # superblocks of 1024 columns


def _split_excess_waits(nc, max_waits=1):
    """The walrus build in this container rejects instructions carrying more
    than `max_waits` sem waits; peel extras onto same-engine NoOps."""
    n_split = 0
    for f in nc.m.functions:
        for bb in f.blocks:
            insts = bb.instructions
            new = []
            changed = False
            for inst in insts:
                si = inst.sync_info
                waits = list(si.on_wait) if si and si.on_wait else []
                if len(waits) > max_waits:
                    changed = True
                    head, keep = waits[:-max_waits], waits[-max_waits:]
                    for i in range(0, len(head), max_waits):
                        nop = mybir.InstNoOp(
                            name=f"wsplit-{inst.name}-{n_split}", ins=[], outs=[])
                        n_split += 1
                        nop.engine = inst.engine
                        nop.sync_info = mybir.SyncInfo(
                            on_wait=head[i:i + max_waits], on_update=[])
                        new.append(nop)
                    inst.sync_info = mybir.SyncInfo(
                        on_wait=keep,
                        on_update=list(si.on_update) if si.on_update else [])
                new.append(inst)
            if changed:
                bb.instructions = new
    return n_split


def _build_nc():
    nc = bass.Bass("TRN2", target_bir_lowering=False, debug=False)

    aps = {}
    for nm, shape, dt in (
        ("xqT", [D, L], BF16), ("xkT", [D, L], BF16), ("xvT", [D, L], BF16),
        ("wqT", [D, DS], BF16), ("wkT", [D, DS], BF16), ("wvT", [D, DS], BF16),
        ("woT", [DS, D], BF16), ("bqk", [128, 4], F32), ("bv", [1, DS], F32),
        ("masks", [4, 128, 512], BF16),
    ):
        aps[nm] = nc.dram_tensor(nm, shape, dt, kind="ExternalInput").ap()
    aps["outT"] = nc.dram_tensor("outT", [D, L], BF16, kind="ExternalOutput").ap()

    with nc.allow_low_precision("bf16 attention intermediates"), \
            tile.TileContext(nc) as tc:
        _emit(nc, tc, aps)

    _split_excess_waits(nc, 1)
    return nc


def _emit(nc, tc, aps):
    mm = nc.tensor.matmul
    ts = bass.ts

    with tc.tile_pool(name="const", bufs=1) as cpool, \
            tc.tile_pool(name="qkv", bufs=1) as qkv:
        # weights: [128, kt, *] so [:, k, m-slice] is a [128, <=512] lhsT
        wq_s = cpool.tile([128, KT, DS], BF16, name="wq_s")
        wk_s = cpool.tile([128, KT, DS], BF16, name="wk_s")
        wv_s = cpool.tile([128, KT, DS], BF16, name="wv_s")
        wo_s = cpool.tile([128, 2, D], BF16, name="wo_s")
        bqk_s = cpool.tile([128, 4], F32, name="bqk_s")
        bv_s = cpool.tile([1, DS], F32, name="bv_s")
        ones_s = cpool.tile([128, 128], F32, name="ones_s")
        ones_r = cpool.tile([128, 64], F32R, name="ones_r")
        mask_s = cpool.tile([128, 4, 512], BF16, name="mask_s")
        bvb_s = cpool.tile([128, DS], BF16, name="bvb_s")

        for t, src in ((wq_s, aps["wqT"]), (wk_s, aps["wkT"]), (wv_s, aps["wvT"])):
            nc.gpsimd.dma_start(out=t[:, :, :],
                                in_=src.rearrange("(k p) m -> p k m", p=128))
        nc.gpsimd.dma_start(out=wo_s[:, :, :],
                            in_=aps["woT"].rearrange("(k p) m -> p k m", p=128))
        nc.gpsimd.dma_start(out=bqk_s[:, :], in_=aps["bqk"])
        nc.gpsimd.dma_start(out=bv_s[:, :], in_=aps["bv"])
        nc.gpsimd.dma_start(out=mask_s[:, :, :],
                            in_=aps["masks"].rearrange("r p j -> p r j"))
        nc.vector.memset(ones_s[:, :], 1.0)
        nc.scalar.copy(ones_r[:, :], ones_s[:, 0:64])

        QT = [qkv.tile([128, L], BF16, name=f"QT{i}") for i in range(2)]
        KTt = [qkv.tile([128, L], BF16, name=f"KTt{i}") for i in range(2)]
        V2 = qkv.tile([128, 16, HPC * 65], BF16, name="V2")
        OTn = [qkv.tile([128, L], BF16, name=f"OTn{i}") for i in range(2)]

        # ones columns of V' (col 64 of each head's 65-wide group): memset the
        # whole tile to 1.0; the V-projection overwrites the 64 value columns.
        nc.vector.memset(V2[:, :, :], 1.0)

        # ---- projections ----
        with tc.tile_pool(name="xs", bufs=1) as xs, \
                tc.tile_pool(name="psA", bufs=1, space="PSUM") as psA:
            # bv broadcast [1,DS] -> [128,DS] via PE ones outer-product
            bvb_ps = psA.tile([128, DS], F32, tag="bvps")
            mm(bvb_ps[:, :], ones_s[0:1, :], bv_s[0:1, :], start=True, stop=True)
            nc.vector.tensor_copy(bvb_s[:, :], bvb_ps[:, :])

            # resident x tiles, one big fully-contiguous DMA each, spread
            # over both HWDGE rings + SWDGE
            xq_t = xs.tile([128, KT, L], BF16, name="xq_t")
            xk_t = xs.tile([128, KT, L], BF16, name="xk_t")
            xv_t = xs.tile([128, KT, L], BF16, name="xv_t")
            for k in range(KT):
                nc.sync.dma_start(
                    out=xq_t[:, k, :],
                    in_=aps["xqT"].rearrange("(k p) l -> p k l", p=128)[:, k, :])
                nc.scalar.dma_start(
                    out=xk_t[:, k, :],
                    in_=aps["xkT"].rearrange("(k p) l -> p k l", p=128)[:, k, :])
                nc.gpsimd.dma_start(
                    out=xv_t[:, k, :],
                    in_=aps["xvT"].rearrange("(k p) l -> p k l", p=128)[:, k, :])

            # Q/K: out[d'128, l512]; k-OUTER accumulation so the PE consumes
            # x k-tiles as the DMAs deliver them (4 psum tiles live per pass)
            for m in range(2):
                for which in range(2):   # 0 = Q, 1 = K
                    w_t = wq_s if which == 0 else wk_s
                    x_t = xq_t if which == 0 else xk_t
                    dst = QT[m] if which == 0 else KTt[m]
                    ps = [psA.tile([128, 512], F32, tag="qk", bufs=4,
                                   name=f"p{which}{m}{n}") for n in range(NCH)]
                    for k in range(KT):
                        for n in range(NCH):
                            mm(ps[n][:, :], w_t[:, k, ts(m, 128)],
                               x_t[:, k, ts(n, 512)],
                               start=(k == 0), stop=(k == KT - 1))
                    for n in range(NCH):
                        nc.vector.tensor_scalar_add(
                            dst[:, ts(n, 512)], ps[n][:, :],
                            bqk_s[:, 2 * which + m:2 * which + m + 1])

            # V: out[l128, d'256], natural layout into V2 (+bias broadcast)
            for lt in range(16):
                pvp = psA.tile([128, DS], F32, tag="v", bufs=3)
                for k in range(KT):
                    mm(pvp[:, :], xv_t[:, k, ts(lt, 128)], wv_s[:, k, :],
                       start=(k == 0), stop=(k == KT - 1))
                nc.vector.tensor_tensor(
                    V2[:, lt:lt + 1, :].rearrange("p o (h c) -> p (o h) c", c=65)[:, :, 0:64],
                    pvp[:, :].rearrange("p (h c) -> p h c", c=64),
                    bvb_s[:, :].rearrange("p (h c) -> p h c", c=64),
                    mybir.AluOpType.add)

        # ---- attention, head by head ----
        with tc.tile_pool(name="att", bufs=1) as att, \
                tc.tile_pool(name="psB", bufs=1, space="PSUM") as psB:
            # deferred normalize: emitted after the NEXT block's first STs so
            # the in-order PE never waits on ACT/DVE
            pending = []

            def flush_pending():
                while pending:
                    pending.pop(0)()

            fin_jobs = []              # deferred final-proj chunks
            for sb in range(NSB):
                for h in range(HPC):
                    po = (h % 2) * 64
                    qt = QT[h // 2]
                    kt_ = KTt[h // 2]
                    base = sb * 2          # first global 512-chunk of superblock
                    pv = psB.tile([65, 1024], F32, tag="pv", bufs=2)
                    nkl = 8 * (sb + 1)

                    sb0 = base * 512       # global q origin of superblock

                    def emit_st(kl):
                        # causal trim at 128 granularity: only q >= kl*128
                        loc0 = max(0, kl * 128 - sb0)
                        st = psB.tile([128, 1024], F32, tag="st", bufs=2,
                                      name="st")
                        est = att.tile([128, 1024], BF16, tag="est", bufs=3,
                                       name="est")
                        for c in range(2):
                            lo, hi = c * 512, (c + 1) * 512
                            lo = max(lo, loc0)
                            if lo >= hi:
                                continue
                            mm(st[:, lo:hi],
                               kt_[po:po + 64, ts(kl, 128)],
                               qt[po:po + 64, sb0 + lo:sb0 + hi],
                               start=True, stop=True)
                        nc.scalar.activation(
                            est[:, loc0:1024], st[:, loc0:1024],
                            mybir.ActivationFunctionType.Exp, scale=0.125)
                        if kl >= 8 * sb:   # diagonal k-tile: mask first 128 cols
                            nc.vector.tensor_tensor(
                                est[:, loc0:loc0 + 128],
                                est[:, loc0:loc0 + 128],
                                mask_s[:, 0, 0:128],
                                mybir.AluOpType.mult)
                        return est

                    def emit_pv(kl, est):
                        loc0 = max(0, kl * 128 - sb0)
                        for c in range(2):
                            lo, hi = c * 512, (c + 1) * 512
                            lo = max(lo, loc0)
                            if lo >= hi:
                                continue
                            mm(pv[:, lo:hi],
                               V2[:, kl, h * 65:(h + 1) * 65],
                               est[:, lo:hi],
                               start=(kl == 0), stop=(kl == nkl - 1),
                               skip_group_check=True)

                    # software pipeline: ST(k+1) is emitted before PV(k) so the
                    # in-order PE never stalls on exp(k); the previous block's
                    # normalize lands between our first STs
                    prev_est = emit_st(0)
                    first = True
                    for kl in range(1, nkl):
                        est = emit_st(kl)
                        if first:
                            flush_pending()
                            first = False
                        emit_pv(kl - 1, prev_est)
                        prev_est = est
                    emit_pv(nkl - 1, prev_est)
                    # sums row copy now (ACT, runs while next block's STs issue)
                    sums = att.tile([65, 1024], F32R, tag="sums", bufs=2)
                    nc.scalar.copy(sums[64:65, :], pv[64:65, :])

                    def normalize(h=h, sb=sb, pv=pv, sums=sums, po=po):
                        bc = psB.tile([64, 1024], F32, tag="st", bufs=2,
                                      name="bc")
                        for c in range(2):
                            mm(bc[:, ts(c, 512)],
                               ones_r[64:65, :],
                               sums[64:65, ts(c, 512)],
                               start=True, stop=True)
                        rec = att.tile([64, 1024], F32, tag="rec", bufs=2,
                                       name="rec")
                        nc.vector.reciprocal(rec[:, :], bc[:, :])
                        if po == 0:
                            nc.vector.tensor_tensor(
                                OTn[h // 2][0:64, ts(sb, 1024)],
                                pv[0:64, :], rec[:, :], mybir.AluOpType.mult)
                        else:
                            osc = att.tile([64, 1024], BF16, tag="osc", bufs=2,
                                           name="osc")
                            nc.vector.tensor_tensor(
                                osc[:, :], pv[0:64, :], rec[:, :],
                                mybir.AluOpType.mult)
                            nc.gpsimd.dma_start(
                                out=OTn[h // 2][64:128, ts(sb, 1024)],
                                in_=osc[:, :])

                    pending.append(normalize)

                    # interleave the first two final-proj column chunks into
                    # sb=1 attention (OTn[:, :1024] is complete by then)
                    if sb == 1 and h in (1, 2):
                        n = h - 1
                        for mt in range(8):
                            opj = psB.tile([128, 512], F32, tag="pv", bufs=2,
                                           name="opj")
                            mm(opj[:, :], wo_s[:, 0, ts(mt, 128)],
                               OTn[0][:, ts(n, 512)], start=True, stop=False)
                            mm(opj[:, :], wo_s[:, 1, ts(mt, 128)],
                               OTn[1][:, ts(n, 512)], start=False, stop=True)
                            obj = att.tile([128, 512], BF16, tag="obi", bufs=3,
                                           name="obj")
                            nc.vector.tensor_copy(obj[:, :], opj[:, :])
                            engj = nc.sync if mt % 2 == 0 else nc.scalar
                            engj.dma_start(
                                out=aps["outT"][mt * 128:(mt + 1) * 128, ts(n, 512)],
                                in_=obj[:, :])
            flush_pending()

        # ---- final projection: partialT[d_out, l] = woT_g^T . OTn ----
        with tc.tile_pool(name="fin", bufs=1) as fin, \
                tc.tile_pool(name="psC", bufs=1, space="PSUM") as psC:
            for mt in range(8):
                for n in range(2, NCH):
                    op_ = psC.tile([128, 512], F32, tag="o", bufs=3)
                    mm(op_[:, :], wo_s[:, 0, ts(mt, 128)], OTn[0][:, ts(n, 512)],
                       start=True, stop=False)
                    mm(op_[:, :], wo_s[:, 1, ts(mt, 128)], OTn[1][:, ts(n, 512)],
                       start=False, stop=True)
                    ob = fin.tile([128, 512], BF16, tag="ob", bufs=4)
                    nc.vector.tensor_copy(ob[:, :], op_[:, :])
                    eng = nc.sync if (mt * NCH + n) % 2 == 0 else nc.scalar
                    eng.dma_start(
                        out=aps["outT"][mt * 128:(mt + 1) * 128, ts(n, 512)],
                        in_=ob[:, :])


_NC_CACHE = None


def _get_nc():
    global _NC_CACHE
    if _NC_CACHE is None:
        _NC_CACHE = _build_nc()
    return _NC_CACHE


def _host_prep(inputs):
    bf16 = ml_dtypes.bfloat16
    q = np.asarray(inputs["query"], np.float32)
    k = np.asarray(inputs["key_"], np.float32)
    v = np.asarray(inputs["value"], np.float32)
    w_q = np.asarray(inputs["w_q"], np.float32)
    w_k = np.asarray(inputs["w_k"], np.float32)
    w_v = np.asarray(inputs["w_v"], np.float32)
    w_o = np.asarray(inputs["w_o"], np.float32)
    b_q = np.asarray(inputs["b_q"], np.float32)
    b_k = np.asarray(inputs["b_k"], np.float32)
    b_v = np.asarray(inputs["b_v"], np.float32)

    # causal diagonal-block masks: mask[r][p, j] = (j - 128*r - p) >= 0
    jj = np.arange(512)[None, None, :]
    pp = np.arange(128)[None, :, None]
    rr = np.arange(4)[:, None, None]
    masks = ((jj - 128 * rr - pp) >= 0).astype(bf16)

    xT = {}
    for b in range(B):
        xT[b] = (
            np.ascontiguousarray(q[b].T).astype(bf16),
            np.ascontiguousarray(k[b].T).astype(bf16),
            np.ascontiguousarray(v[b].T).astype(bf16),
        )

    in_maps = []
    for c in range(N_CORES):
        b, g = divmod(c, 4)
        sl = slice(g * DS, (g + 1) * DS)
        bqk = np.stack([
            b_q[sl][0:128], b_q[sl][128:256],
            b_k[sl][0:128], b_k[sl][128:256],
        ], axis=1).astype(np.float32)            # [128, 4]
        in_maps.append({
            "xqT": xT[b][0], "xkT": xT[b][1], "xvT": xT[b][2],
            "wqT": np.ascontiguousarray(w_q[sl, :].T).astype(bf16),
            "wkT": np.ascontiguousarray(w_k[sl, :].T).astype(bf16),
            "wvT": np.ascontiguousarray(w_v[sl, :].T).astype(bf16),
            "woT": np.ascontiguousarray(w_o[:, sl].T).astype(bf16),
            "bqk": bqk,
            "bv": b_v[sl].reshape(1, DS).astype(np.float32),
            "masks": masks,
        })
    return in_maps


def kernel(**inputs):
    nc = _get_nc()
    in_maps = _host_prep(inputs)
    res = run_bass_kernel_spmd(
        nc, in_maps, core_ids=list(range(N_CORES)), trace=False)
    b_o = np.asarray(inputs["b_o"], np.float32)
    out = np.empty((B, L, D), np.float32)
    for b in range(B):
        acc = np.zeros((D, L), np.float32)
        for g in range(4):
            acc += res.results[b * 4 + g]["outT"].astype(np.float32)
        out[b] = acc.T + b_o
    return out



# revision 45
# speedup vs baseline: 18.7165x; 18.7165x over previous
"""Multi-head causal attention (B=2, L=2048, D=1024, H=16) on 8 trn2 cores.

Sharding: core c -> batch b=c//4, head-group g=c%4 (4 heads / 256 of D).

v2 schedule: software-pipelined across phases.
 - Input x loaded in L-halves; head phase projects Q/K (m=0,1) + V(lt 0..3)
   for the first L-half with k-outer accumulation so the PE consumes k-tiles
   as DMAs land.
 - Attention runs over 4 q-superblocks of 512 with the scores transposed
   (st[k, q]); PV uses est chunks as lhsT producing pv[q, d'] so the
   softmax denominators become per-partition scalars (cheap DVE
   tensor_scalar) and PV costs 65 cols instead of 128 per (kl, q-chunk).
 - Remaining projections (L1 halves, V lt 4..15) and the final w_o
   projection are folded into the attention slots as PE filler while ACT
   digests exp.
 - Attention output [q, d'] is PE-transposed per superblock into OTn[d', q]
   for the final projection; fin chunks for superblock n become filler
   once its transposes are emitted.
"""

import sys

sys.path.insert(0, "/opt/trn_rl_repo")

import numpy as np
import ml_dtypes

import concourse.bass as bass
import concourse.mybir as mybir
import concourse.tile as tile
from concourse.bass_utils import run_bass_kernel_spmd
from concourse.masks import make_identity

BF16 = mybir.dt.bfloat16
F32 = mybir.dt.float32

DEBUG_OUTS = False

B, L, D, H = 2, 2048, 1024, 16
DK = 64            # head dim
HPC = 4            # heads per core
DS = HPC * DK      # 256: D-slice per core
KT = D // 128      # 8 k-tiles over D
N_CORES = 8
NSB = 4            # q-superblocks of 512
SBW = 512


def _split_excess_waits(nc, max_waits=1):
    """The walrus build in this container rejects instructions carrying more
    than `max_waits` sem waits; peel extras onto same-engine NoOps."""
    n_split = 0
    for f in nc.m.functions:
        for bb in f.blocks:
            insts = bb.instructions
            new = []
            changed = False
            for inst in insts:
                si = inst.sync_info
                waits = list(si.on_wait) if si and si.on_wait else []
                if len(waits) > max_waits:
                    changed = True
                    head, keep = waits[:-max_waits], waits[-max_waits:]
                    for i in range(0, len(head), max_waits):
                        nop = mybir.InstNoOp(
                            name=f"wsplit-{inst.name}-{n_split}", ins=[], outs=[])
                        n_split += 1
                        nop.engine = inst.engine
                        nop.sync_info = mybir.SyncInfo(
                            on_wait=head[i:i + max_waits], on_update=[])
                        new.append(nop)
                    inst.sync_info = mybir.SyncInfo(
                        on_wait=keep,
                        on_update=list(si.on_update) if si.on_update else [])
                new.append(inst)
            if changed:
                bb.instructions = new
    return n_split


def _build_nc(n_iters=1):
    nc = bass.Bass("TRN2", target_bir_lowering=False, debug=False)

    aps = {}
    for nm, shape, dt in (
        ("xqT", [D, L], BF16), ("xkT", [D, L], BF16), ("xvT", [D, L], BF16),
        ("wqT", [D, DS], BF16), ("wkT", [D, DS], BF16), ("wvT", [D, DS], BF16),
        ("woT", [DS, D], BF16), ("bqk", [128, 4], F32), ("bv", [1, DS], F32),
        ("masks", [4, 128, 512], BF16),
    ):
        aps[nm] = nc.dram_tensor(nm, shape, dt, kind="ExternalInput").ap()
    aps["outT"] = nc.dram_tensor("outT", [D, L], BF16, kind="ExternalOutput").ap()
    if DEBUG_OUTS:
        for nm, shape in (("dQT0", [128, L]), ("dKT0", [128, L]),
                          ("dV2", [128, 16 * HPC * 65]),
                          ("dOT0", [128, L]), ("dOT1", [128, L])):
            aps[nm] = nc.dram_tensor(nm, shape, BF16,
                                     kind="ExternalOutput").ap()

    with nc.allow_low_precision("bf16 attention intermediates"), \
            tile.TileContext(nc) as tc:
        for _ in range(n_iters):
            _emit(nc, tc, aps)

    _split_excess_waits(nc, 1)
    return nc


def _emit(nc, tc, aps):
    mm = nc.tensor.matmul
    ts = bass.ts

    with tc.tile_pool(name="const", bufs=1) as cpool, \
            tc.tile_pool(name="qkv", bufs=1) as qkv, \
            tc.tile_pool(name="xs", bufs=1) as xs, \
            tc.tile_pool(name="att", bufs=1) as att, \
            tc.tile_pool(name="ps", bufs=1, space="PSUM") as ps:

        # ---- const tiles ----
        wq_s = cpool.tile([128, KT, DS], BF16, name="wq_s")
        wk_s = cpool.tile([128, KT, DS], BF16, name="wk_s")
        wv_s = cpool.tile([128, KT, DS], BF16, name="wv_s")
        wo_s = cpool.tile([128, 2, D], BF16, name="wo_s")
        bqk_s = cpool.tile([128, 4], F32, name="bqk_s")
        bv_s = cpool.tile([1, DS], F32, name="bv_s")
        mask_s = cpool.tile([128, 128], BF16, name="mask_s")
        ident = cpool.tile([128, 128], BF16, name="ident")
        ones_s = cpool.tile([1, 128], F32, name="ones_s")
        bvb_s = cpool.tile([128, DS], BF16, name="bvb_s")

        # wq/wk lead (first k-steps need them); everything else interleaves
        # with the quarter loads below.
        nc.sync.dma_start(out=wq_s[:, :, :],
                          in_=aps["wqT"].rearrange("(k p) m -> p k m", p=128))
        nc.scalar.dma_start(out=wk_s[:, :, :],
                            in_=aps["wkT"].rearrange("(k p) m -> p k m", p=128))
        nc.gpsimd.dma_start(out=bqk_s[:, :], in_=aps["bqk"])
        nc.gpsimd.dma_start(out=bv_s[:, :], in_=aps["bv"])
        nc.gpsimd.dma_start(
            out=mask_s[:, :],
            in_=aps["masks"].rearrange("r p j -> p r j")[:, 0, 0:128])
        make_identity(nc, ident[:, :])
        nc.vector.memset(ones_s[:, :], 1.0)

        # ---- x tiles: one DMA per (tensor, L-quarter) ----
        # 1KB partition lines keep full DMA bandwidth, the FIFO transfer
        # queue naturally delivers quarter q before quarter q+1, and the
        # attention superblock q only needs x columns < (q+1)*512.  xq-Q0 is
        # k-halved so the first projection k-steps start ~2us earlier.
        xq_t = xs.tile([128, KT, L], BF16, name="xq_t")
        xk_t = xs.tile([128, KT, L], BF16, name="xk_t")
        xv_t = xs.tile([128, KT, L], BF16, name="xv_t")
        xq_r = aps["xqT"].rearrange("(k p) l -> p k l", p=128)
        xk_r = aps["xkT"].rearrange("(k p) l -> p k l", p=128)
        xv_r = aps["xvT"].rearrange("(k p) l -> p k l", p=128)
        q0 = slice(0, 512)
        nc.sync.dma_start(out=xq_t[:, 0:4, q0], in_=xq_r[:, 0:4, q0])
        nc.scalar.dma_start(out=xk_t[:, 0:4, q0], in_=xk_r[:, 0:4, q0])
        nc.sync.dma_start(out=xq_t[:, 4:8, q0], in_=xq_r[:, 4:8, q0])
        nc.scalar.dma_start(out=xk_t[:, 4:8, q0], in_=xk_r[:, 4:8, q0])
        nc.sync.dma_start(out=xv_t[:, :, q0], in_=xv_r[:, :, q0])
        nc.scalar.dma_start(out=wv_s[:, :, :],
                            in_=aps["wvT"].rearrange("(k p) m -> p k m", p=128))
        for q in range(1, 4):
            qs = slice(q * 512, (q + 1) * 512)
            nc.sync.dma_start(out=xq_t[:, :, qs], in_=xq_r[:, :, qs])
            nc.scalar.dma_start(out=xk_t[:, :, qs], in_=xk_r[:, :, qs])
            nc.sync.dma_start(out=xv_t[:, :, qs], in_=xv_r[:, :, qs])
            if q == 1:
                nc.scalar.dma_start(
                    out=wo_s[:, :, :],
                    in_=aps["woT"].rearrange("(k p) m -> p k m", p=128))

        # ---- persistent activations ----
        QT = [qkv.tile([128, L], BF16, name=f"QT{i}") for i in range(2)]
        KTt = [qkv.tile([128, L], BF16, name=f"KTt{i}") for i in range(2)]
        V2 = qkv.tile([128, 16, HPC * 65], BF16, name="V2")
        OTn = [qkv.tile([128, L], BF16, name=f"OTn{i}") for i in range(2)]

        # ones columns of V' (col 64 of each head's 65-wide group)
        nc.vector.memset(
            V2[:, :, :].rearrange("p l (h c) -> p l h c", c=65)[:, :, :, 64:65],
            1.0)

        # bv broadcast [1,DS] -> [128,DS] via PE ones outer-product
        bvb_ps = ps.tile([128, 512], F32, tag="proj", bufs=2, name="bvb_ps")
        mm(bvb_ps[:, 0:DS], ones_s[0:1, :], bv_s[0:1, :], start=True, stop=True)
        nc.vector.tensor_copy(bvb_s[:, :], bvb_ps[:, 0:DS])

        # ---------- chunk emitters ----------
        def qk_bias(dst, t, n, bi):
            nc.vector.tensor_scalar_add(
                dst[:, ts(n, 512)], t[:, :], bqk_s[:, bi:bi + 1])

        def v_write(t, lt):
            nc.vector.tensor_tensor(
                V2[:, lt:lt + 1, :]
                .rearrange("p o (h c) -> p (o h) c", c=65)[:, :, 0:64],
                t[:, 0:DS].rearrange("p (h c) -> p h c", c=64),
                bvb_s[:, :].rearrange("p (h c) -> p h c", c=64),
                mybir.AluOpType.add)

        def emit_qk_kouter(specs):
            """specs: list of (which, m, n); k-outer over all chunks."""
            tags = ["proj", "st", "proj", "st"]
            tiles = [ps.tile([128, 512], F32, tag=tags[i % 4], bufs=2,
                              name=f"hqk{i}")
                     for i in range(len(specs))]
            for k in range(KT):
                for t, (which, m, n) in zip(tiles, specs):
                    w_t = wq_s if which == 0 else wk_s
                    x_t = xq_t if which == 0 else xk_t
                    mm(t[:, :], w_t[:, k, ts(m, 128)], x_t[:, k, ts(n, 512)],
                       start=(k == 0), stop=(k == KT - 1))
            for t, (which, m, n) in zip(tiles, specs):
                dst = (QT if which == 0 else KTt)[m]
                qk_bias(dst, t, n, 2 * which + m)

        def emit_v_kouter(lts):
            tags = ["proj", "st"]
            tiles = [ps.tile([128, 512], F32, tag=tags[i % 2], bufs=2,
                              name=f"hv{i}")
                     for i in range(len(lts))]
            for k in range(KT):
                for t, lt in zip(tiles, lts):
                    mm(t[:, 0:DS], xv_t[:, k, ts(lt, 128)], wv_s[:, k, :],
                       start=(k == 0), stop=(k == KT - 1))
            for t, lt in zip(tiles, lts):
                v_write(t, lt)

        # ---------- filler queue (labeled, with prerequisite barriers) ----
        fillq = []           # list of (label, fn)
        emitted = set()      # labels whose LAST unit has been emitted

        def pump(n):
            for _ in range(n):
                if fillq:
                    lbl, fn = fillq.pop(0)
                    fn()
                    if lbl is not None:
                        emitted.add(lbl)

        def require(labels):
            """Drain the queue until every label in `labels` is emitted."""
            queued = {lbl for lbl, _ in fillq} | emitted
            for lbl in labels:
                assert lbl in queued, f"filler barrier on unqueued {lbl}"
            while not all(lbl in emitted for lbl in labels):
                pump(1)

        def queue_qk_chunk(which, m, n):
            state = {}
            lbl = f"qk{which}{m}{n}"

            def unit(k):
                def f():
                    if k == 0:
                        state["t"] = ps.tile([128, 512], F32, tag="proj",
                                             bufs=2, name="fqk")
                    w_t = wq_s if which == 0 else wk_s
                    x_t = xq_t if which == 0 else xk_t
                    mm(state["t"][:, :], w_t[:, k, ts(m, 128)],
                       x_t[:, k, ts(n, 512)],
                       start=(k == 0), stop=(k == KT - 1))
                    if k == KT - 1:
                        dst = (QT if which == 0 else KTt)[m]
                        qk_bias(dst, state["t"], n, 2 * which + m)
                return f

            for k in range(KT):
                fillq.append((lbl if k == KT - 1 else None, unit(k)))

        def queue_v_chunk(lt):
            state = {}
            lbl = f"v{lt}"

            def unit(k):
                def f():
                    if k == 0:
                        state["t"] = ps.tile([128, 512], F32, tag="proj",
                                             bufs=2, name="fv")
                    mm(state["t"][:, 0:DS], xv_t[:, k, ts(lt, 128)],
                       wv_s[:, k, :], start=(k == 0), stop=(k == KT - 1))
                    if k == KT - 1:
                        v_write(state["t"], lt)
                return f

            for k in range(KT):
                fillq.append((lbl if k == KT - 1 else None, unit(k)))

        fin_count = [0]

        def fin_units(mt, n, tail=False, tag="proj"):
            """(start, finish) units for one fin chunk.  start reads OTn[0]
            (ready after this superblock's half-0 transposes), finish reads
            OTn[1], stops the accumulation and ships the result."""
            state = {}

            def u0():
                state["t"] = ps.tile([128, 512], F32, tag=tag, bufs=2,
                                     name="ffin")
                mm(state["t"][:, :], wo_s[:, 0, ts(mt, 128)],
                   OTn[0][:, ts(n, 512)], start=True, stop=False)

            def u1():
                mm(state["t"][:, :], wo_s[:, 1, ts(mt, 128)],
                   OTn[1][:, ts(n, 512)], start=False, stop=True)
                ob = att.tile([128, 512], BF16, tag="ob", bufs=3, name="ob")
                i = fin_count[0]
                fin_count[0] += 1
                if tail and i % 2 == 1:
                    # ACT is idle once the last exp retires; SP ring beats
                    # SWDGE's ~2us gen+sem latency for the closing DMAs
                    nc.scalar.copy(ob[:, :], state["t"][:, :])
                    nc.scalar.dma_start(
                        out=aps["outT"][mt * 128:(mt + 1) * 128, ts(n, 512)],
                        in_=ob[:, :])
                else:
                    nc.vector.tensor_copy(ob[:, :], state["t"][:, :])
                    eng = nc.sync if (tail or i % 2 == 0) else nc.gpsimd
                    eng.dma_start(
                        out=aps["outT"][mt * 128:(mt + 1) * 128, ts(n, 512)],
                        in_=ob[:, :])

            return u0, u1

        # ---------- head phase (before attention) ----------
        # Only the m=0, n=0 projection chunks run ahead of attention —
        # heads 0,1 of superblock 0 need nothing else from Q/K.
        emit_qk_kouter([(0, 0, 0), (1, 0, 0)])

        # filler backlog, ordered by when attention needs the results
        queue_qk_chunk(0, 1, 0)
        queue_qk_chunk(1, 1, 0)
        for lt in (0, 1, 2, 3):
            queue_v_chunk(lt)
        for n in (1, 2, 3):
            for m in range(2):
                queue_qk_chunk(0, m, n)
                queue_qk_chunk(1, m, n)
            for lt in range(4 * n, 4 * n + 4):
                queue_v_chunk(lt)

        # prerequisites per superblock (in-order PE: anything an attention
        # block reads must be EMITTED before the reader, else deadlock)
        # at sb start only heads 0,1 run, so only the m=0 chunks gate it;
        # m=1 gates h2, and V chunks are required lazily per k-tile
        prereq = {s: [f"qk{w}0{s}" for w in range(2)] for s in (1, 2, 3)}
        prereq_h2 = {s: [f"qk{w}1{s}" for w in range(2)] for s in (0, 1, 2, 3)}

        def emit_transposes(AT, sb0, half):
            for c in range(4):
                tp = ps.tile([128, 128], BF16, tag="tp", bufs=2, name="tp")
                nc.tensor.transpose(
                    tp[:, :], AT[:, c, ts(half, 128)], ident[:, :])
                dst = OTn[half][:, sb0 + c * 128:sb0 + (c + 1) * 128]
                nc.vector.tensor_copy(dst, tp[:, :])
                pump(1)

        # pump pacing: save filler (esp. fin chunks) for the later, bigger
        # superblocks where ACT-bound exp leaves the PE underfed
        rates = [2, 1, 1, 2]
        pend_fins = []

        # ---------- attention ----------
        for sb in range(NSB):
            sb0 = sb * SBW
            require(prereq.get(sb, []))
            AT = att.tile([128, 4, DS], BF16, tag="AT", bufs=2, name="AT")
            for h in range(HPC):
                po = (h % 2) * 64
                qt = QT[h // 2]
                kt_ = KTt[h // 2]
                nkl = 4 * (sb + 1)
                # padded to a full 2KB PSUM bank so no other tile shares
                # this zero region
                pv = ps.tile([128, 4, 128], F32, tag="pv", bufs=2, name="pv")

                def emit_st(kl):
                    cmin = max(0, kl - 4 * sb)
                    loc0 = cmin * 128
                    st = ps.tile([128, 512], F32, tag="st", bufs=2, name="st")
                    mm(st[:, loc0:512], kt_[po:po + 64, ts(kl, 128)],
                       qt[po:po + 64, sb0 + loc0:sb0 + 512],
                       start=True, stop=True)
                    est = att.tile([128, 512], BF16, tag="est", bufs=5, name="est")
                    nc.scalar.activation(
                        est[:, loc0:512], st[:, loc0:512],
                        mybir.ActivationFunctionType.Exp, scale=0.125)
                    if kl >= 4 * sb:
                        nc.vector.tensor_tensor(
                            est[:, loc0:loc0 + 128], est[:, loc0:loc0 + 128],
                            mask_s[:, :], mybir.AluOpType.mult)
                    return est

                def emit_pv(kl, est):
                    # ONE start for the whole PSUM zero region: start=True
                    # marks the full 2KB region pending-zero, so each chunk's
                    # first write overwrites and later writes accumulate.
                    # Additional start=True flags would re-mark sibling
                    # chunks' bytes and wipe their partial sums.
                    cmin = max(0, kl - 4 * sb)
                    for c in range(cmin, 4):
                        mm(pv[:, c, 0:65], est[:, ts(c, 128)],
                           V2[:, kl, h * 65:(h + 1) * 65],
                           start=(kl == 0 and c == 0),
                           stop=(kl == 4 * sb + c),
                           skip_group_check=True)

                if h == 2:
                    require(prereq_h2.get(sb, []))
                if sb == 0:
                    # burst all 4 STs first: exp (ACT) starts as soon as the
                    # n0 projections finish, while the PVs wait for the V
                    # chunks still in flight; filler pumps between
                    ests = []
                    for kl in range(nkl):
                        ests.append(emit_st(kl))
                        pump(2)
                    for kl in range(nkl):
                        require([f"v{kl}"])
                        emit_pv(kl, ests[kl])
                        pump(1)
                else:
                    prev = emit_st(0)
                    for kl in range(1, nkl):
                        est = emit_st(kl)
                        pump(rates[sb])
                        if kl - 1 >= 4 * sb:
                            require([f"v{kl - 1}"])
                        emit_pv(kl - 1, prev)
                        prev = est
                    pump(1)
                    require([f"v{nkl - 1}"])
                    emit_pv(nkl - 1, prev)

                # normalize: per-partition reciprocal of the sums column,
                # then scale the 64 value columns per q-chunk (DVE only)
                rec = att.tile([128, 4], F32, tag="rec", bufs=2, name="rec")
                nc.vector.reciprocal(
                    rec[:, :], pv[:, :, 64:65].rearrange("p c o -> p (c o)"))
                for c in range(4):
                    nc.vector.tensor_scalar_mul(
                        AT[:, c, h * 64:(h + 1) * 64], pv[:, c, 0:64],
                        rec[:, c:c + 1])

                if h == 1:
                    # heads 0,1 fill AT cols 0:128 -> OTn[0]; transpose now
                    # and unlock the first two fin starts for this sb
                    emit_transposes(AT, sb0, 0)
                    tail = sb == NSB - 1
                    tags = (["proj", "proj", "st", "st", "proj", "proj",
                             "st", "st"] if tail else ["proj"] * 8)
                    fu = [fin_units(mt, sb, tail=tail, tag=tags[mt])
                          for mt in range(8)]
                    fillq.append((None, fu[0][0]))
                    fillq.append((None, fu[1][0]))
                    pend_fins.append(fu)
                elif h == 3:
                    emit_transposes(AT, sb0, 1)
                    fu = pend_fins.pop()
                    if sb == NSB - 1:
                        # attention is over: the "st" psum bufs are free, so
                        # run the closing chunks 4 wide to shorten the tail
                        for i in (0, 1):
                            fillq.append((None, fu[i][1]))
                        for i in (2, 3, 4, 5):
                            fillq.append((None, fu[i][0]))
                        for i in (2, 3):
                            fillq.append((None, fu[i][1]))
                        for i in (6, 7):
                            fillq.append((None, fu[i][0]))
                        for i in (4, 5, 6, 7):
                            fillq.append((None, fu[i][1]))
                    else:
                        # ping-pong starts and finishes: at most 2 open
                        # accumulations hold the 2 "proj" psum bufs
                        for mt in range(2, 8):
                            fillq.append((None, fu[mt - 2][1]))
                            fillq.append((None, fu[mt][0]))
                        fillq.append((None, fu[6][1]))
                        fillq.append((None, fu[7][1]))

        # ---------- drain ----------
        while fillq:
            pump(1)

        if DEBUG_OUTS:
            nc.gpsimd.dma_start(out=aps["dQT0"], in_=QT[0][:, :])
            nc.gpsimd.dma_start(out=aps["dKT0"], in_=KTt[0][:, :])
            nc.gpsimd.dma_start(
                out=aps["dV2"],
                in_=V2[:, :, :].rearrange("p l c -> p (l c)"))
            nc.gpsimd.dma_start(out=aps["dOT0"], in_=OTn[0][:, :])
            nc.gpsimd.dma_start(out=aps["dOT1"], in_=OTn[1][:, :])


_NC_CACHE = None


def _get_nc():
    global _NC_CACHE
    if _NC_CACHE is None:
        _NC_CACHE = _build_nc()
    return _NC_CACHE


def _host_prep(inputs):
    bf16 = ml_dtypes.bfloat16
    q = np.asarray(inputs["query"], np.float32)
    k = np.asarray(inputs["key_"], np.float32)
    v = np.asarray(inputs["value"], np.float32)
    w_q = np.asarray(inputs["w_q"], np.float32)
    w_k = np.asarray(inputs["w_k"], np.float32)
    w_v = np.asarray(inputs["w_v"], np.float32)
    w_o = np.asarray(inputs["w_o"], np.float32)
    b_q = np.asarray(inputs["b_q"], np.float32)
    b_k = np.asarray(inputs["b_k"], np.float32)
    b_v = np.asarray(inputs["b_v"], np.float32)

    # causal diagonal-block masks: mask[r][p, j] = (j - 128*r - p) >= 0
    jj = np.arange(512)[None, None, :]
    pp = np.arange(128)[None, :, None]
    rr = np.arange(4)[:, None, None]
    masks = ((jj - 128 * rr - pp) >= 0).astype(bf16)

    xT = {}
    for b in range(B):
        xT[b] = (
            np.ascontiguousarray(q[b].T).astype(bf16),
            np.ascontiguousarray(k[b].T).astype(bf16),
            np.ascontiguousarray(v[b].T).astype(bf16),
        )

    in_maps = []
    for c in range(N_CORES):
        b, g = divmod(c, 4)
        sl = slice(g * DS, (g + 1) * DS)
        bqk = np.stack([
            b_q[sl][0:128], b_q[sl][128:256],
            b_k[sl][0:128], b_k[sl][128:256],
        ], axis=1).astype(np.float32)            # [128, 4]
        in_maps.append({
            "xqT": xT[b][0], "xkT": xT[b][1], "xvT": xT[b][2],
            "wqT": np.ascontiguousarray(w_q[sl, :].T).astype(bf16),
            "wkT": np.ascontiguousarray(w_k[sl, :].T).astype(bf16),
            "wvT": np.ascontiguousarray(w_v[sl, :].T).astype(bf16),
            "woT": np.ascontiguousarray(w_o[:, sl].T).astype(bf16),
            "bqk": bqk,
            "bv": b_v[sl].reshape(1, DS).astype(np.float32),
            "masks": masks,
        })
    return in_maps


def kernel(**inputs):
    nc = _get_nc()
    in_maps = _host_prep(inputs)
    res = run_bass_kernel_spmd(
        nc, in_maps, core_ids=list(range(N_CORES)), trace=False)
    b_o = np.asarray(inputs["b_o"], np.float32)
    out = np.empty((B, L, D), np.float32)
    for b in range(B):
        acc = np.zeros((D, L), np.float32)
        for g in range(4):
            acc += res.results[b * 4 + g]["outT"].astype(np.float32)
        out[b] = acc.T + b_o
    return out


# revision 55
# speedup vs baseline: 88.3701x; 4.7215x over previous
"""Multi-head causal attention (B=2, L=2048, D=1024, H=16) on 8 trn2 cores.

Sharding: core c -> batch b=c//4, head-group g=c%4 (4 heads / 256 of D).
Host sums the 4 per-group partials per batch (+ b_o) during unsharding.

Schedule (fully software-pipelined; TimelineSim ~141us/core vs ~204us for
the phase-sequential v1):
 - x loaded via one DMA per (tensor, L-quarter) at 1KB partition lines; the
   FIFO transfer queue delivers quarter q before q+1, matching when
   attention superblock q needs it.  Only the m0/n0 Q,K projection chunks
   run ahead of attention; all other projection chunks are filler units.
 - Attention runs over 4 q-superblocks of 512 with transposed scores
   (st[k, q]); PV uses est chunks as lhsT producing pv[q, d'], so PV costs
   65 output cols instead of 128 per (kl, q-chunk) and the softmax
   denominators become per-partition scalars (one DVE reciprocal + 4
   tensor_scalar_muls per head-block, no PE broadcast).
 - The 4 q-chunk accumulators share one PSUM zero region: exactly ONE
   start_tensor_calc=True (kl=0, c=0) marks the 2KB region pending-zero;
   every chunk's first write then overwrites and later writes accumulate.
   More start flags would re-mark sibling bytes and wipe partial sums.
 - Remaining projections and the w_o projection drain from a labeled filler
   queue into attention slots while ACT digests exp; require() barriers
   force-drain anything a block reads before its reader is emitted
   (in-order PE would deadlock otherwise).
 - Attention out [q, d'] is PE-transposed per superblock half (after h1 and
   h3) into OTn[d', q]; fin chunks ship per superblock, and the last
   superblock's fins run 4-wide over the freed st/proj PSUM banks to
   shorten the tail.
"""

import sys

sys.path.insert(0, "/opt/trn_rl_repo")

import numpy as np
import ml_dtypes

import concourse.bass as bass
import concourse.mybir as mybir
import concourse.tile as tile
from concourse.bass_utils import run_bass_kernel_spmd
from concourse.masks import make_identity

BF16 = mybir.dt.bfloat16
F32 = mybir.dt.float32

DEBUG_OUTS = False

B, L, D, H = 2, 2048, 1024, 16
DK = 64            # head dim
HPC = 4            # heads per core
DS = HPC * DK      # 256: D-slice per core
KT = D // 128      # 8 k-tiles over D
N_CORES = 8
NSB = 4            # q-superblocks of 512
SBW = 512


def _split_excess_waits(nc, max_waits=1):
    """The walrus build in this container rejects instructions carrying more
    than `max_waits` sem waits; peel extras onto same-engine NoOps."""
    n_split = 0
    for f in nc.m.functions:
        for bb in f.blocks:
            insts = bb.instructions
            new = []
            changed = False
            for inst in insts:
                si = inst.sync_info
                waits = list(si.on_wait) if si and si.on_wait else []
                if len(waits) > max_waits:
                    changed = True
                    head, keep = waits[:-max_waits], waits[-max_waits:]
                    for i in range(0, len(head), max_waits):
                        nop = mybir.InstNoOp(
                            name=f"wsplit-{inst.name}-{n_split}", ins=[], outs=[])
                        n_split += 1
                        nop.engine = inst.engine
                        nop.sync_info = mybir.SyncInfo(
                            on_wait=head[i:i + max_waits], on_update=[])
                        new.append(nop)
                    inst.sync_info = mybir.SyncInfo(
                        on_wait=keep,
                        on_update=list(si.on_update) if si.on_update else [])
                new.append(inst)
            if changed:
                bb.instructions = new
    return n_split


def _build_nc(n_iters=1):
    nc = bass.Bass("TRN2", target_bir_lowering=False, debug=False)

    aps = {}
    for nm, shape, dt in (
        ("xqT", [D, L], BF16), ("xkT", [D, L], BF16), ("xvT", [D, L], BF16),
        ("wqT", [D, DS], BF16), ("wkT", [D, DS], BF16), ("wvT", [D, DS], BF16),
        ("woT", [DS, D], BF16), ("bqk", [128, 4], F32), ("bv", [1, DS], F32),
        ("masks", [4, 128, 512], BF16),
    ):
        aps[nm] = nc.dram_tensor(nm, shape, dt, kind="ExternalInput").ap()
    aps["outT"] = nc.dram_tensor("outT", [D, L], BF16, kind="ExternalOutput").ap()
    if DEBUG_OUTS:
        for nm, shape in (("dQT0", [128, L]), ("dKT0", [128, L]),
                          ("dV2", [128, 16 * HPC * 65]),
                          ("dOT0", [128, L]), ("dOT1", [128, L])):
            aps[nm] = nc.dram_tensor(nm, shape, BF16,
                                     kind="ExternalOutput").ap()

    with nc.allow_low_precision("bf16 attention intermediates"), \
            tile.TileContext(nc) as tc:
        for _ in range(n_iters):
            _emit(nc, tc, aps)

    _split_excess_waits(nc, 1)
    return nc


def _emit(nc, tc, aps):
    mm = nc.tensor.matmul
    ts = bass.ts

    with tc.tile_pool(name="const", bufs=1) as cpool, \
            tc.tile_pool(name="qkv", bufs=1) as qkv, \
            tc.tile_pool(name="xs", bufs=1) as xs, \
            tc.tile_pool(name="att", bufs=1) as att, \
            tc.tile_pool(name="ps", bufs=1, space="PSUM") as ps:

        # ---- const tiles ----
        wq_s = cpool.tile([128, KT, DS], BF16, name="wq_s")
        wk_s = cpool.tile([128, KT, DS], BF16, name="wk_s")
        wv_s = cpool.tile([128, KT, DS], BF16, name="wv_s")
        wo_s = cpool.tile([128, 2, D], BF16, name="wo_s")
        bqk_s = cpool.tile([128, 4], F32, name="bqk_s")
        bv_s = cpool.tile([1, DS], F32, name="bv_s")
        mask_s = cpool.tile([128, 128], BF16, name="mask_s")
        ident = cpool.tile([128, 128], BF16, name="ident")
        ones_s = cpool.tile([1, 128], F32, name="ones_s")
        bvb_s = cpool.tile([128, DS], BF16, name="bvb_s")

        # wq/wk k-halves interleave with the x quarter-loads so the first
        # projection k-steps start as early as possible
        nc.sync.dma_start(out=wq_s[:, 0:4, :],
                          in_=aps["wqT"].rearrange("(k p) m -> p k m",
                                                   p=128)[:, 0:4, :])
        nc.scalar.dma_start(out=wk_s[:, 0:4, :],
                            in_=aps["wkT"].rearrange("(k p) m -> p k m",
                                                     p=128)[:, 0:4, :])
        nc.gpsimd.dma_start(out=bqk_s[:, :], in_=aps["bqk"])
        nc.gpsimd.dma_start(out=bv_s[:, :], in_=aps["bv"])
        nc.gpsimd.dma_start(
            out=mask_s[:, :],
            in_=aps["masks"].rearrange("r p j -> p r j")[:, 0, 0:128])
        make_identity(nc, ident[:, :])
        nc.vector.memset(ones_s[:, :], 1.0)

        # ---- x tiles: one DMA per (tensor, L-quarter) ----
        # 1KB partition lines keep full DMA bandwidth, the FIFO transfer
        # queue naturally delivers quarter q before quarter q+1, and the
        # attention superblock q only needs x columns < (q+1)*512.  xq-Q0 is
        # k-halved so the first projection k-steps start ~2us earlier.
        xq_t = xs.tile([128, KT, L], BF16, name="xq_t")
        xk_t = xs.tile([128, KT, L], BF16, name="xk_t")
        xv_t = xs.tile([128, KT, L], BF16, name="xv_t")
        xq_r = aps["xqT"].rearrange("(k p) l -> p k l", p=128)
        xk_r = aps["xkT"].rearrange("(k p) l -> p k l", p=128)
        xv_r = aps["xvT"].rearrange("(k p) l -> p k l", p=128)
        q0 = slice(0, 512)
        nc.sync.dma_start(out=xq_t[:, 0:4, q0], in_=xq_r[:, 0:4, q0])
        nc.scalar.dma_start(out=xk_t[:, 0:4, q0], in_=xk_r[:, 0:4, q0])
        nc.sync.dma_start(out=wq_s[:, 4:8, :],
                          in_=aps["wqT"].rearrange("(k p) m -> p k m",
                                                   p=128)[:, 4:8, :])
        nc.scalar.dma_start(out=wk_s[:, 4:8, :],
                            in_=aps["wkT"].rearrange("(k p) m -> p k m",
                                                     p=128)[:, 4:8, :])
        nc.sync.dma_start(out=xq_t[:, 4:8, q0], in_=xq_r[:, 4:8, q0])
        nc.scalar.dma_start(out=xk_t[:, 4:8, q0], in_=xk_r[:, 4:8, q0])
        nc.sync.dma_start(out=xv_t[:, :, q0], in_=xv_r[:, :, q0])
        nc.scalar.dma_start(out=wv_s[:, :, :],
                            in_=aps["wvT"].rearrange("(k p) m -> p k m", p=128))
        for q in range(1, 4):
            qs = slice(q * 512, (q + 1) * 512)
            nc.sync.dma_start(out=xq_t[:, :, qs], in_=xq_r[:, :, qs])
            nc.scalar.dma_start(out=xk_t[:, :, qs], in_=xk_r[:, :, qs])
            nc.sync.dma_start(out=xv_t[:, :, qs], in_=xv_r[:, :, qs])
            if q == 1:
                nc.scalar.dma_start(
                    out=wo_s[:, :, :],
                    in_=aps["woT"].rearrange("(k p) m -> p k m", p=128))

        # ---- persistent activations ----
        QT = [qkv.tile([128, L], BF16, name=f"QT{i}") for i in range(2)]
        KTt = [qkv.tile([128, L], BF16, name=f"KTt{i}") for i in range(2)]
        V2 = qkv.tile([128, 16, HPC * 65], BF16, name="V2")
        OTn = [qkv.tile([128, L], BF16, name=f"OTn{i}") for i in range(2)]

        # ones columns of V' (col 64 of each head's 65-wide group)
        nc.vector.memset(
            V2[:, :, :].rearrange("p l (h c) -> p l h c", c=65)[:, :, :, 64:65],
            1.0)

        # bv broadcast [1,DS] -> [128,DS] via PE ones outer-product
        bvb_ps = ps.tile([128, 512], F32, tag="proj", bufs=2, name="bvb_ps")
        mm(bvb_ps[:, 0:DS], ones_s[0:1, :], bv_s[0:1, :], start=True, stop=True)
        nc.vector.tensor_copy(bvb_s[:, :], bvb_ps[:, 0:DS])

        # ---------- chunk emitters ----------
        def qk_bias(dst, t, n, bi):
            nc.vector.tensor_scalar_add(
                dst[:, ts(n, 512)], t[:, :], bqk_s[:, bi:bi + 1])

        def v_write(t, lt):
            nc.vector.tensor_tensor(
                V2[:, lt:lt + 1, :]
                .rearrange("p o (h c) -> p (o h) c", c=65)[:, :, 0:64],
                t[:, 0:DS].rearrange("p (h c) -> p h c", c=64),
                bvb_s[:, :].rearrange("p (h c) -> p h c", c=64),
                mybir.AluOpType.add)

        def emit_qk_kouter(specs):
            """specs: list of (which, m, n); k-outer over all chunks."""
            tags = ["proj", "st", "proj", "st"]
            tiles = [ps.tile([128, 512], F32, tag=tags[i % 4],
                              bufs=(3 if tags[i % 4] == "st" else 2),
                              name=f"hqk{i}")
                     for i in range(len(specs))]
            for k in range(KT):
                for t, (which, m, n) in zip(tiles, specs):
                    w_t = wq_s if which == 0 else wk_s
                    x_t = xq_t if which == 0 else xk_t
                    mm(t[:, :], w_t[:, k, ts(m, 128)], x_t[:, k, ts(n, 512)],
                       start=(k == 0), stop=(k == KT - 1))
            for t, (which, m, n) in zip(tiles, specs):
                dst = (QT if which == 0 else KTt)[m]
                qk_bias(dst, t, n, 2 * which + m)

        def emit_v_kouter(lts):
            tags = ["proj", "st"]
            tiles = [ps.tile([128, 512], F32, tag=tags[i % 2],
                              bufs=(3 if tags[i % 2] == "st" else 2),
                              name=f"hv{i}")
                     for i in range(len(lts))]
            for k in range(KT):
                for t, lt in zip(tiles, lts):
                    mm(t[:, 0:DS], xv_t[:, k, ts(lt, 128)], wv_s[:, k, :],
                       start=(k == 0), stop=(k == KT - 1))
            for t, lt in zip(tiles, lts):
                v_write(t, lt)

        # ---------- filler queue (labeled, with prerequisite barriers) ----
        fillq = []           # list of (label, fn)
        emitted = set()      # labels whose LAST unit has been emitted

        def pump(n):
            for _ in range(n):
                if fillq:
                    lbl, fn = fillq.pop(0)
                    fn()
                    if lbl is not None:
                        emitted.add(lbl)

        def require(labels):
            """Drain the queue until every label in `labels` is emitted."""
            queued = {lbl for lbl, _ in fillq} | emitted
            for lbl in labels:
                assert lbl in queued, f"filler barrier on unqueued {lbl}"
            while not all(lbl in emitted for lbl in labels):
                pump(1)

        def queue_qk_chunk(which, m, n):
            state = {}
            lbl = f"qk{which}{m}{n}"

            def unit(k):
                def f():
                    if k == 0:
                        state["t"] = ps.tile([128, 512], F32, tag="proj",
                                             bufs=2, name="fqk")
                    w_t = wq_s if which == 0 else wk_s
                    x_t = xq_t if which == 0 else xk_t
                    mm(state["t"][:, :], w_t[:, k, ts(m, 128)],
                       x_t[:, k, ts(n, 512)],
                       start=(k == 0), stop=(k == KT - 1))
                    if k == KT - 1:
                        dst = (QT if which == 0 else KTt)[m]
                        qk_bias(dst, state["t"], n, 2 * which + m)
                return f

            for k in range(KT):
                fillq.append((lbl if k == KT - 1 else None, unit(k)))

        def queue_v_chunk(lt):
            state = {}
            lbl = f"v{lt}"

            def unit(k):
                def f():
                    if k == 0:
                        state["t"] = ps.tile([128, 512], F32, tag="proj",
                                             bufs=2, name="fv")
                    mm(state["t"][:, 0:DS], xv_t[:, k, ts(lt, 128)],
                       wv_s[:, k, :], start=(k == 0), stop=(k == KT - 1))
                    if k == KT - 1:
                        v_write(state["t"], lt)
                return f

            for k in range(KT):
                fillq.append((lbl if k == KT - 1 else None, unit(k)))

        fin_count = [0]

        def fin_units(mt, n, tail=False, tag="proj"):
            """(start, finish) units for one fin chunk.  start reads OTn[0]
            (ready after this superblock's half-0 transposes), finish reads
            OTn[1], stops the accumulation and ships the result."""
            state = {}

            def u0():
                state["t"] = ps.tile([128, 512], F32, tag=tag,
                                     bufs=(3 if tag == "st" else 2),
                                     name="ffin")
                mm(state["t"][:, :], wo_s[:, 0, ts(mt, 128)],
                   OTn[0][:, ts(n, 512)], start=True, stop=False)

            def u1():
                mm(state["t"][:, :], wo_s[:, 1, ts(mt, 128)],
                   OTn[1][:, ts(n, 512)], start=False, stop=True)
                ob = att.tile([128, 512], BF16, tag="ob", bufs=8, name="ob")
                i = fin_count[0]
                fin_count[0] += 1
                if tail and i % 2 == 1:
                    # ACT is idle once the last exp retires; SP ring beats
                    # SWDGE's ~2us gen+sem latency for the closing DMAs
                    nc.scalar.copy(ob[:, :], state["t"][:, :])
                    nc.scalar.dma_start(
                        out=aps["outT"][mt * 128:(mt + 1) * 128, ts(n, 512)],
                        in_=ob[:, :])
                else:
                    nc.vector.tensor_copy(ob[:, :], state["t"][:, :])
                    eng = nc.sync if (tail or i % 2 == 0) else nc.gpsimd
                    eng.dma_start(
                        out=aps["outT"][mt * 128:(mt + 1) * 128, ts(n, 512)],
                        in_=ob[:, :])

            return u0, u1

        # ---------- head phase (before attention) ----------
        # Only the m=0, n=0 projection chunks run ahead of attention —
        # heads 0,1 of superblock 0 need nothing else from Q/K.
        emit_qk_kouter([(0, 0, 0), (1, 0, 0)])

        # filler backlog, ordered by when attention needs the results
        queue_qk_chunk(0, 1, 0)
        queue_qk_chunk(1, 1, 0)
        for lt in (0, 1, 2, 3):
            queue_v_chunk(lt)
        for n in (1, 2, 3):
            for m in range(2):
                queue_qk_chunk(0, m, n)
                queue_qk_chunk(1, m, n)
            for lt in range(4 * n, 4 * n + 4):
                queue_v_chunk(lt)

        # prerequisites per superblock (in-order PE: anything an attention
        # block reads must be EMITTED before the reader, else deadlock)
        # at sb start only heads 0,1 run, so only the m=0 chunks gate it;
        # m=1 gates h2, and V chunks are required lazily per k-tile
        prereq = {s: [f"qk{w}0{s}" for w in range(2)] for s in (1, 2, 3)}
        prereq_h2 = {s: [f"qk{w}1{s}" for w in range(2)] for s in (0, 1, 2, 3)}

        def emit_transposes(AT, sb0, half):
            # all 4 transposes of a half share one PSUM bank-tile (one
            # region start; each later write overwrites its pending bytes);
            # bufs=1 is safe because consecutive halves are 2 head-blocks
            # apart, far past the copies
            tp = ps.tile([128, 4, 128], BF16, tag="tp", bufs=1, name="tp")
            for c in range(4):
                mm(tp[:, c, :], AT[:, c, ts(half, 128)], ident[:, :],
                   is_transpose=True, start=(c == 0), stop=(c == 3),
                   skip_group_check=True)
                dst = OTn[half][:, sb0 + c * 128:sb0 + (c + 1) * 128]
                nc.vector.tensor_copy(dst, tp[:, c, :])
                pump(1)

        # pump pacing: save filler (esp. fin chunks) for the later, bigger
        # superblocks where ACT-bound exp leaves the PE underfed
        rates = [2, 1, 1, 2]
        pend_fins = []

        # ---------- attention ----------
        sb0_ests = {}

        def emit_st_for(hh, kl):
            po_ = (hh % 2) * 64
            qt_ = QT[hh // 2]
            kt2 = KTt[hh // 2]
            loc0 = kl * 128
            st = ps.tile([128, 512], F32, tag="st", bufs=3, name="st")
            mm(st[:, loc0:512], kt2[po_:po_ + 64, ts(kl, 128)],
               qt_[po_:po_ + 64, loc0:512], start=True, stop=True)
            est = att.tile([128, 512], BF16, tag="est", bufs=18, name="est")
            nc.scalar.activation(
                est[:, loc0:512], st[:, loc0:512],
                mybir.ActivationFunctionType.Exp, scale=0.125)
            nc.vector.tensor_tensor(
                est[:, loc0:loc0 + 128], est[:, loc0:loc0 + 128],
                mask_s[:, :], mybir.AluOpType.mult)
            return est

        for sb in range(NSB):
            sb0 = sb * SBW
            require(prereq.get(sb, []))
            AT = att.tile([128, 4, DS], BF16, tag="AT", bufs=2, name="AT")
            if sb == 0:
                # burst ALL 16 superblock-0 STs: a continuous exp stream for
                # ACT right out of the projection head phase, while the PVs
                # wait for the V chunks still in flight
                for hh in range(HPC):
                    if hh == 2:
                        require(prereq_h2.get(0, []))
                    sb0_ests[hh] = []
                    for kl in range(4):
                        sb0_ests[hh].append(emit_st_for(hh, kl))
                        pump(2)
            for h in range(HPC):
                po = (h % 2) * 64
                qt = QT[h // 2]
                kt_ = KTt[h // 2]
                nkl = 4 * (sb + 1)
                # padded to a full 2KB PSUM bank so no other tile shares
                # this zero region
                pv = ps.tile([128, 4, 128], F32, tag="pv", bufs=2, name="pv")

                def emit_st(kl):
                    cmin = max(0, kl - 4 * sb)
                    loc0 = cmin * 128
                    st = ps.tile([128, 512], F32, tag="st", bufs=3, name="st")
                    mm(st[:, loc0:512], kt_[po:po + 64, ts(kl, 128)],
                       qt[po:po + 64, sb0 + loc0:sb0 + 512],
                       start=True, stop=True)
                    est = att.tile([128, 512], BF16, tag="est", bufs=18, name="est")
                    nc.scalar.activation(
                        est[:, loc0:512], st[:, loc0:512],
                        mybir.ActivationFunctionType.Exp, scale=0.125)
                    if kl >= 4 * sb:
                        nc.vector.tensor_tensor(
                            est[:, loc0:loc0 + 128], est[:, loc0:loc0 + 128],
                            mask_s[:, :], mybir.AluOpType.mult)
                    return est

                def emit_pv(kl, est):
                    # ONE start for the whole PSUM zero region: start=True
                    # marks the full 2KB region pending-zero, so each chunk's
                    # first write overwrites and later writes accumulate.
                    # Additional start=True flags would re-mark sibling
                    # chunks' bytes and wipe their partial sums.
                    cmin = max(0, kl - 4 * sb)
                    for c in range(cmin, 4):
                        mm(pv[:, c, 0:65], est[:, ts(c, 128)],
                           V2[:, kl, h * 65:(h + 1) * 65],
                           start=(kl == 0 and c == 0),
                           stop=(kl == 4 * sb + c),
                           skip_group_check=True)

                if h == 2:
                    require(prereq_h2.get(sb, []))
                if sb == 0:
                    for kl in range(nkl):
                        require([f"v{kl}"])
                        emit_pv(kl, sb0_ests[h][kl])
                        pump(1)
                else:
                    prev = emit_st(0)
                    for kl in range(1, nkl):
                        est = emit_st(kl)
                        pump(rates[sb])
                        if kl - 1 >= 4 * sb:
                            require([f"v{kl - 1}"])
                        emit_pv(kl - 1, prev)
                        prev = est
                    pump(1)
                    require([f"v{nkl - 1}"])
                    emit_pv(nkl - 1, prev)

                # normalize: per-partition reciprocal of the sums column,
                # then scale the 64 value columns per q-chunk (DVE only)
                rec = att.tile([128, 4], F32, tag="rec", bufs=4, name="rec")
                nc.vector.reciprocal(
                    rec[:, :], pv[:, :, 64:65].rearrange("p c o -> p (c o)"))
                for c in range(4):
                    nc.vector.tensor_scalar_mul(
                        AT[:, c, h * 64:(h + 1) * 64], pv[:, c, 0:64],
                        rec[:, c:c + 1])

                if h == 1:
                    # heads 0,1 fill AT cols 0:128 -> OTn[0]; transpose now
                    # and unlock the first two fin starts for this sb
                    emit_transposes(AT, sb0, 0)
                    tail = sb == NSB - 1
                    tags = (["proj", "proj", "st", "st", "proj", "proj",
                             "st", "st"] if tail else ["proj"] * 8)
                    fu = [fin_units(mt, sb, tail=tail, tag=tags[mt])
                          for mt in range(8)]
                    fillq.append((None, fu[0][0]))
                    fillq.append((None, fu[1][0]))
                    pend_fins.append(fu)
                elif h == 3:
                    emit_transposes(AT, sb0, 1)
                    fu = pend_fins.pop()
                    if sb == NSB - 1:
                        # attention is over: the "st" psum bufs are free, so
                        # run the closing chunks 4 wide to shorten the tail
                        for i in (0, 1):
                            fillq.append((None, fu[i][1]))
                        for i in (2, 3, 4, 5):
                            fillq.append((None, fu[i][0]))
                        for i in (2, 3):
                            fillq.append((None, fu[i][1]))
                        for i in (6, 7):
                            fillq.append((None, fu[i][0]))
                        for i in (4, 5, 6, 7):
                            fillq.append((None, fu[i][1]))
                    else:
                        # ping-pong starts and finishes: at most 2 open
                        # accumulations hold the 2 "proj" psum bufs
                        for mt in range(2, 8):
                            fillq.append((None, fu[mt - 2][1]))
                            fillq.append((None, fu[mt][0]))
                        fillq.append((None, fu[6][1]))
                        fillq.append((None, fu[7][1]))

        # ---------- drain ----------
        while fillq:
            pump(1)

        if DEBUG_OUTS:
            nc.gpsimd.dma_start(out=aps["dQT0"], in_=QT[0][:, :])
            nc.gpsimd.dma_start(out=aps["dKT0"], in_=KTt[0][:, :])
            nc.gpsimd.dma_start(
                out=aps["dV2"],
                in_=V2[:, :, :].rearrange("p l c -> p (l c)"))
            nc.gpsimd.dma_start(out=aps["dOT0"], in_=OTn[0][:, :])
            nc.gpsimd.dma_start(out=aps["dOT1"], in_=OTn[1][:, :])


_NC_CACHE = None


def _get_nc():
    global _NC_CACHE
    if _NC_CACHE is None:
        _NC_CACHE = _build_nc()
    return _NC_CACHE


def _host_prep(inputs):
    bf16 = ml_dtypes.bfloat16
    q = np.asarray(inputs["query"], np.float32)
    k = np.asarray(inputs["key_"], np.float32)
    v = np.asarray(inputs["value"], np.float32)
    w_q = np.asarray(inputs["w_q"], np.float32)
    w_k = np.asarray(inputs["w_k"], np.float32)
    w_v = np.asarray(inputs["w_v"], np.float32)
    w_o = np.asarray(inputs["w_o"], np.float32)
    b_q = np.asarray(inputs["b_q"], np.float32)
    b_k = np.asarray(inputs["b_k"], np.float32)
    b_v = np.asarray(inputs["b_v"], np.float32)

    # causal diagonal-block masks: mask[r][p, j] = (j - 128*r - p) >= 0
    jj = np.arange(512)[None, None, :]
    pp = np.arange(128)[None, :, None]
    rr = np.arange(4)[:, None, None]
    masks = ((jj - 128 * rr - pp) >= 0).astype(bf16)

    xT = {}
    for b in range(B):
        xT[b] = (
            np.ascontiguousarray(q[b].T).astype(bf16),
            np.ascontiguousarray(k[b].T).astype(bf16),
            np.ascontiguousarray(v[b].T).astype(bf16),
        )

    in_maps = []
    for c in range(N_CORES):
        b, g = divmod(c, 4)
        sl = slice(g * DS, (g + 1) * DS)
        bqk = np.stack([
            b_q[sl][0:128], b_q[sl][128:256],
            b_k[sl][0:128], b_k[sl][128:256],
        ], axis=1).astype(np.float32)            # [128, 4]
        in_maps.append({
            "xqT": xT[b][0], "xkT": xT[b][1], "xvT": xT[b][2],
            "wqT": np.ascontiguousarray(w_q[sl, :].T).astype(bf16),
            "wkT": np.ascontiguousarray(w_k[sl, :].T).astype(bf16),
            "wvT": np.ascontiguousarray(w_v[sl, :].T).astype(bf16),
            "woT": np.ascontiguousarray(w_o[:, sl].T).astype(bf16),
            "bqk": bqk,
            "bv": b_v[sl].reshape(1, DS).astype(np.float32),
            "masks": masks,
        })
    return in_maps


def kernel(**inputs):
    nc = _get_nc()
    in_maps = _host_prep(inputs)
    res = run_bass_kernel_spmd(
        nc, in_maps, core_ids=list(range(N_CORES)), trace=False)
    b_o = np.asarray(inputs["b_o"], np.float32)
    out = np.empty((B, L, D), np.float32)
    for b in range(B):
        acc = np.zeros((D, L), np.float32)
        for g in range(4):
            acc += res.results[b * 4 + g]["outT"].astype(np.float32)
        out[b] = acc.T + b_o
    return out
